# revision 1
# baseline (speedup 1.0000x reference)
"""GraphMAE-style GNN forward (3x GCNConv + BN + PReLU + SCE loss) on 8 TRN2
NeuronCores via Bass/Tile.

Sharding: nodes partitioned across 8 cores (6272 rows each, node space padded
to 50176). Per layer: sharded dense matmul (feature-major) -> row scale by
dinv -> AllGather of the bf16 node-major table -> per-dst-tile edge
aggregation via indirect-DMA row gathers + one-hot segment-sum matmuls
accumulated in PSUM (self-loops folded in as identity matmuls on the local
shard tile) -> BatchNorm batch stats via free-dim reductions + AllReduce ->
fused BN affine + PReLU (as max(t, a*t), valid for 0<a<=1). Loss: per-core
masked-row cosine-similarity partial sums, AllReduce, 1 - sum/NMASK.

Per-feature conv bias is skipped: training-mode BatchNorm subtracts the batch
mean, so the bias cancels exactly.
"""
import sys

sys.path.insert(0, "/opt/trn_rl_repo")
import numpy as np
import ml_dtypes

import concourse.bass as bass
import concourse.mybir as mybir
import concourse.tile as tile
from bass_rust import SyncInfo
from concourse.bass import IndirectOffsetOnAxis
from concourse.bass_utils import run_bass_kernel_spmd
from concourse.tile_rust import add_dep_helper
from concourse.vector_clock import ScopedClock

BF = ml_dtypes.bfloat16
F32 = mybir.dt.float32
BF16 = mybir.dt.bfloat16
I32 = mybir.dt.int32
OP = mybir.AluOpType

N, E, IN, HID, OUT = 50000, 800000, 128, 256, 128
NC, P = 8, 128
SHARD, T = 6272, 49
NPAD = NC * SHARD
NMASK = 25000
EPS = 1e-5
RG = [list(range(NC))]

# ---------------------------------------------------------------------------
# Walrus on this toolchain rejects >1 semaphore wait per instruction
# ("Too many sync wait commands"). Post-process lowered blocks: hoist excess
# waits onto injected same-engine NoOps (program order => equivalent).
_uid = [0]


def _split_bb_waits(nc):
    for f in nc.m.functions:
        for bb in f.blocks:
            insts = list(bb.instructions)
            out = []
            changed = False
            for inst in insts:
                si = inst.sync_info
                waits = list(si.on_wait) if si is not None and si.on_wait else []
                if len(waits) > 1:
                    changed = True
                    rest = waits[:-1]
                    inst.sync_info.on_wait = waits[-1:]
                    while rest:
                        _uid[0] += 1
                        nop = mybir.InstNoOp(
                            name=f"I-waitsplit-{_uid[0]}", ins=[], outs=[])
                        nop.engine = inst.engine
                        nop.sync_info = SyncInfo(
                            on_wait=rest[:1], on_update=[])
                        rest = rest[1:]
                        out.append(nop)
                out.append(inst)
            if changed:
                bb.instructions = out


class TileContextSplitDrain(tile.TileContext):
    def _drain_and_barrier(self, tick_clock, wait_clock):
        nc = self.nc
        probe = nc.sync.nop(nofuse=True)
        wait_clock.add_sem_waits(
            probe.ins, ScopedClock({None: tick_clock.global_clock}))
        nc.sync.drain()
        nc.all_engine_barrier()
        assert self.sems is not None
        popped = nc._tile_sem_poison_stack.pop()
        assert popped is self._sem_poison
        nc.clear_and_free_semaphores(list(self.sems.allocated().values()))
        nc.all_engine_barrier()
        _split_bb_waits(nc)


# ---------------------------------------------------------------------------
def _prep(edge_index, mask_nodes):
    src = edge_index[0].astype(np.int64)
    dst = edge_index[1].astype(np.int64)
    deg = np.bincount(dst, minlength=N).astype(np.float64) + 1.0
    dinv = (1.0 / np.sqrt(deg)).astype(np.float32)
    dinv_pad = np.zeros(NPAD, np.float32)
    dinv_pad[:N] = dinv

    mask_set = np.zeros(N, bool)
    mask_set[mask_nodes] = True

    pertile = []
    kc = 0
    core_of = dst // SHARD
    for c in range(NC):
        sel = core_of == c
        es, ed = src[sel], dst[sel] - c * SHARD
        tl = ed // P
        order = np.argsort(tl, kind="stable")
        es, ed, tl = es[order], ed[order], tl[order]
        tiles = []
        for t in range(T):
            m = tl == t
            tiles.append((es[m], (ed[m] % P).astype(np.float32)))
            kc = max(kc, (int(m.sum()) + P - 1) // P)
        pertile.append(tiles)

    srcs, dstl = [], []
    for c in range(NC):
        sc = np.zeros((T * kc, P), np.int32)
        dc = np.full((T * kc, P), 255.0, np.float32)
        for t in range(T):
            es, dl = pertile[c][t]
            n = len(es)
            sc[t * kc:(t + 1) * kc].reshape(-1)[:n] = es
            dc[t * kc:(t + 1) * kc].reshape(-1)[:n] = dl
        srcs.append(np.ascontiguousarray(sc.T))
        dstl.append(np.ascontiguousarray(dc.T.astype(BF)))

    mlocal, mglob, mvalid, mkeep = [], [], [], []
    locs = []
    mc = 0
    for c in range(NC):
        rows = np.arange(c * SHARD, (c + 1) * SHARD)
        valid = rows < N
        mm = np.zeros(SHARD, bool)
        mm[:valid.sum()] = mask_set[rows[valid]]
        loc = np.where(mm)[0]
        mc = max(mc, (len(loc) + P - 1) // P)
        locs.append(loc)
        keep = np.ones(SHARD, np.float32)
        keep[mm] = 0.0
        mkeep.append(keep)
    for c in range(NC):
        loc = locs[c]
        n = len(loc)
        lo = np.zeros((mc, P), np.int32)
        gl = np.zeros((mc, P), np.int32)
        va = np.zeros((mc, P), np.float32)
        lo.reshape(-1)[:n] = loc
        gl.reshape(-1)[:n] = loc + c * SHARD
        va.reshape(-1)[:n] = 1.0
        mlocal.append(np.ascontiguousarray(lo.T))
        mglob.append(np.ascontiguousarray(gl.T))
        mvalid.append(np.ascontiguousarray(va.T))
    return dinv_pad, srcs, dstl, kc, mlocal, mglob, mvalid, mkeep, mc


def build_nc(kc, mc):
    nc = bass.Bass(num_devices=NC, target_bir_lowering=False)
    D = {}

    def inp(name, shape, dt):
        D[name] = nc.dram_tensor(name, shape, dt, kind="ExternalInput")
        return D[name]

    x_shard = inp("x_shard", [SHARD, IN], F32)
    x_full = inp("x_full", [N, IN], F32)
    inp("src_rows", [P, T * kc], I32)
    inp("dst_local", [P, T * kc], BF16)
    inp("dinv_rep", [P, SHARD], F32)
    inp("mcol", [P, T], F32)
    inp("token_rep", [P, IN], F32)
    inp("iota_bf", [P, P], BF16)
    inp("ident_bf", [P, P], BF16)
    inp("ident_f32", [P, P], F32)
    inp("ones_col", [P, 1], F32)
    inp("w1", [IN, HID], BF16)
    inp("w2a", [P, OUT], BF16)
    inp("w2b", [P, OUT], BF16)
    inp("wd", [OUT, IN], BF16)
    inp("gb", [P, 8], F32)
    inp("a_rep", [P, 3], F32)
    inp("mrow_l", [P, mc], I32)
    inp("mrow_g", [P, mc], I32)
    inp("mval", [P, mc], F32)
    out_t = nc.dram_tensor("loss", [1, 1], F32, kind="ExternalOutput")

    with TileContextSplitDrain(nc) as tc:
        with (
            tc.tile_pool(name="const", bufs=1) as cpool,
            tc.tile_pool(name="hbuf", bufs=1) as hpool,
            tc.tile_pool(name="vbuf", bufs=1) as vpool,
            tc.tile_pool(name="hwn", bufs=1) as hwpool,
            tc.tile_pool(name="work", bufs=2) as wk,
            tc.tile_pool(name="gpool", bufs=4) as gp,
            tc.tile_pool(name="psmm", bufs=2, space="PSUM") as psmm,
            tc.tile_pool(name="pstr", bufs=1, space="PSUM") as pstr,
            tc.tile_pool(name="psagg", bufs=1, space="PSUM") as psagg,
            tc.tile_pool(name="dram", bufs=1, space="DRAM") as dpool,
        ):
            def load(tname):
                h = D[tname]
                t_ = cpool.tile(list(h.shape), h.dtype, tag=tname, name=tname)
                nc.sync.dma_start(t_[:], h[:])
                return t_

            src_s = load("src_rows")
            dstl_s = load("dst_local")
            dinv_s = load("dinv_rep")
            mcol_s = load("mcol")
            tokr_s = load("token_rep")
            iota_s = load("iota_bf")
            idbf_s = load("ident_bf")
            idf_s = load("ident_f32")
            ones_s = load("ones_col")
            w1_s = load("w1")
            w2a_s = load("w2a")
            w2b_s = load("w2b")
            wd_s = load("wd")
            gb_s = load("gb")
            a_s = load("a_rep")
            mrl_s = load("mrow_l")
            mrg_s = load("mrow_g")
            mv_s = load("mval")

            # h0: load x shard, mask in node-major, transpose to f-major bf16
            h0 = hpool.tile([P, SHARD], BF16, tag="hA_0", name="hA_0")
            for b in range(T):
                xt = wk.tile([P, IN], F32, tag="xt", name="xt")
                nc.sync.dma_start(xt[:], x_shard[b * P:(b + 1) * P, :])
                nc.vector.tensor_tensor(out=xt[:], in0=xt[:], in1=tokr_s[:],
                                        op=OP.subtract)
                nc.vector.tensor_scalar(out=xt[:], in0=xt[:],
                                        scalar1=mcol_s[:, b:b + 1], scalar2=None,
                                        op0=OP.mult)
                nc.vector.tensor_tensor(out=xt[:], in0=xt[:], in1=tokr_s[:],
                                        op=OP.add)
                pt = pstr.tile([P, P], F32, tag="ptr32", name="ptr32")
                nc.tensor.transpose(pt[:], xt[:], idf_s[:])
                nc.vector.tensor_copy(out=h0[:, b * P:(b + 1) * P], in_=pt[:])

            table1 = dpool.tile([NPAD, HID], BF16, addr_space="Shared", name="table1")
            table2 = dpool.tile([NPAD, OUT], BF16, addr_space="Shared", name="table2")
            table3 = dpool.tile([NPAD, IN], BF16, addr_space="Shared", name="table3")
            agin1 = dpool.tile([SHARD, HID], BF16, name="agin1")
            agin2 = dpool.tile([SHARD, OUT], BF16, name="agin2")
            agin3 = dpool.tile([SHARD, IN], BF16, name="agin3")
            loss_in = dpool.tile([1, 16], F32, name="loss_in")
            loss_out = dpool.tile([1, 16], F32, addr_space="Shared", name="loss_out")
            rex_dram = dpool.tile([SHARD, IN], F32, name="rex_dram")

            def layer(h_tiles, w_tiles, f_in, f_out, agin, table, g_col,
                      be_col, a_col, otag, mask_hwn):
                nfh = f_out // P
                nkt = f_in // P
                hwn = [hwpool.tile([P, f_out], BF16, tag=f"hwn_{b}", name=f"hwn_{b}")
                       for b in range(T)]
                for b in range(T):
                    for fh in range(nfh):
                        pm = psmm.tile([P, P], F32, tag="pm", name="pm")
                        for kt in range(nkt):
                            nc.tensor.matmul(
                                pm[:], lhsT=w_tiles[kt][:, fh * P:(fh + 1) * P],
                                rhs=h_tiles[kt][:, b * P:(b + 1) * P],
                                start=(kt == 0), stop=(kt == nkt - 1))
                        hwT = wk.tile([P, P], BF16, tag="hwT", name="hwT")
                        nc.vector.tensor_tensor(
                            out=hwT[:], in0=pm[:],
                            in1=dinv_s[:, b * P:(b + 1) * P], op=OP.mult)
                        ptb = pstr.tile([P, P], BF16, tag="ptrbf", name="ptrbf")
                        nc.tensor.transpose(ptb[:], hwT[:], idbf_s[:])
                        sl = hwn[b][:, fh * P:(fh + 1) * P]
                        if mask_hwn:
                            nc.vector.tensor_scalar(
                                out=sl, in0=ptb[:], scalar1=mcol_s[:, b:b + 1],
                                scalar2=None, op0=OP.mult)
                        else:
                            nc.vector.tensor_copy(out=sl, in_=ptb[:])
                    nc.sync.dma_start(agin[b * P:(b + 1) * P, :], hwn[b][:])
                cc = nc.gpsimd.collective_compute(
                    "AllGather", OP.bypass, replica_groups=RG,
                    ins=[agin[:].opt()], outs=[table[:].opt()])
                fence = nc.gpsimd.memset(
                    wk.tile([1, 1], F32, tag="fence", name="fence")[:], 0.0)
                add_dep_helper(fence.ins, cc.ins, True, "fence cc")

                v_tiles = [vpool.tile([P, SHARD], BF16, tag=f"v_{fh}", name=f"v_{fh}")
                           for fh in range(nfh)]
                s_cols = [wk.tile([P, T], F32, tag=f"sc_{fh}", name=f"sc_{fh}")
                          for fh in range(nfh)]
                q_cols = [wk.tile([P, T], F32, tag=f"qc_{fh}", name=f"qc_{fh}")
                          for fh in range(nfh)]
                for t in range(T):
                    pa = [psagg.tile([P, P], F32, tag=f"pa{fh}", name=f"pa{fh}")
                          for fh in range(nfh)]
                    for fh in range(nfh):
                        nc.tensor.matmul(
                            pa[fh][:], lhsT=hwn[t][:, fh * P:(fh + 1) * P],
                            rhs=idbf_s[:], start=True, stop=False)
                    for k in range(kc):
                        j = t * kc + k
                        G = gp.tile([P, f_out], BF16, tag="G", name="G")
                        nc.gpsimd.indirect_dma_start(
                            out=G[:], out_offset=None, in_=table[:],
                            in_offset=IndirectOffsetOnAxis(
                                ap=src_s[:, j:j + 1], axis=0))
                        S = gp.tile([P, P], BF16, tag="S", name="S")
                        nc.vector.tensor_tensor(
                            out=S[:],
                            in0=dstl_s[:, j:j + 1].to_broadcast([P, P]),
                            in1=iota_s[:], op=OP.is_equal)
                        for fh in range(nfh):
                            nc.tensor.matmul(
                                pa[fh][:], lhsT=G[:, fh * P:(fh + 1) * P],
                                rhs=S[:], start=False, stop=(k == kc - 1))
                    for fh in range(nfh):
                        vt = v_tiles[fh]
                        sl = slice(t * P, (t + 1) * P)
                        nc.vector.tensor_tensor(
                            out=vt[:, sl], in0=pa[fh][:],
                            in1=dinv_s[:, sl], op=OP.mult)
                        nc.vector.tensor_reduce(
                            out=s_cols[fh][:, t:t + 1], in_=vt[:, sl],
                            axis=mybir.AxisListType.X, op=OP.add)
                        sq = wk.tile([P, P], F32, tag="sq", name="sq")
                        nc.vector.tensor_tensor(
                            out=sq[:], in0=vt[:, sl], in1=vt[:, sl],
                            op=OP.mult)
                        nc.vector.tensor_reduce(
                            out=q_cols[fh][:, t:t + 1], in_=sq[:],
                            axis=mybir.AxisListType.X, op=OP.add)

                stats_in = dpool.tile([P, 4], F32, name=f"stats_in_{otag}")
                stats_out = dpool.tile([P, 4], F32, addr_space="Shared",
                                       name=f"stats_out_{otag}")
                st = wk.tile([P, 4], F32, tag="stats", name="stats")
                nc.gpsimd.memset(st[:], 0.0)
                for fh in range(nfh):
                    nc.vector.tensor_reduce(
                        out=st[:, fh:fh + 1], in_=s_cols[fh][:],
                        axis=mybir.AxisListType.X, op=OP.add)
                    nc.vector.tensor_reduce(
                        out=st[:, 2 + fh:3 + fh], in_=q_cols[fh][:],
                        axis=mybir.AxisListType.X, op=OP.add)
                nc.sync.dma_start(stats_in[:], st[:])
                cc2 = nc.gpsimd.collective_compute(
                    "AllReduce", OP.add, replica_groups=RG,
                    ins=[stats_in[:].opt()], outs=[stats_out[:].opt()])
                st2 = wk.tile([P, 4], F32, tag="stats2", name="stats2")
                ld2 = nc.sync.dma_start(st2[:], stats_out[:])
                add_dep_helper(ld2.ins, cc2.ins, True, "stats after ar")
                h_out = [hpool.tile([P, SHARD], BF16, tag=f"{otag}_{fh}", name=f"{otag}_{fh}")
                         for fh in range(nfh)]
                AB = []
                for fh in range(nfh):
                    mu = wk.tile([P, 1], F32, tag=f"mu{fh}", name=f"mu{fh}")
                    nc.vector.tensor_scalar(out=mu[:], in0=st2[:, fh:fh + 1],
                                            scalar1=1.0 / N, scalar2=None,
                                            op0=OP.mult)
                    var = wk.tile([P, 1], F32, tag=f"var{fh}", name=f"var{fh}")
                    nc.vector.tensor_tensor(out=var[:], in0=mu[:], in1=mu[:],
                                            op=OP.mult)
                    ms = wk.tile([P, 1], F32, tag=f"ms{fh}", name=f"ms{fh}")
                    nc.vector.tensor_scalar(out=ms[:], in0=st2[:, 2 + fh:3 + fh],
                                            scalar1=1.0 / N, scalar2=None,
                                            op0=OP.mult)
                    nc.vector.tensor_tensor(out=var[:], in0=ms[:], in1=var[:],
                                            op=OP.subtract)
                    nc.vector.tensor_scalar(out=var[:], in0=var[:], scalar1=EPS,
                                            scalar2=None, op0=OP.add)
                    rs = wk.tile([P, 1], F32, tag=f"rs{fh}", name=f"rs{fh}")
                    nc.scalar.activation(rs[:], var[:],
                                         mybir.ActivationFunctionType.Sqrt)
                    nc.vector.reciprocal(rs[:], rs[:])
                    A = wk.tile([P, 1], F32, tag=f"A{fh}", name=f"A{fh}")
                    nc.vector.tensor_tensor(out=A[:], in0=g_col[fh], in1=rs[:],
                                            op=OP.mult)
                    Bv = wk.tile([P, 1], F32, tag=f"B{fh}", name=f"B{fh}")
                    nc.vector.tensor_tensor(out=Bv[:], in0=mu[:], in1=A[:],
                                            op=OP.mult)
                    nc.vector.tensor_tensor(out=Bv[:], in0=be_col[fh], in1=Bv[:],
                                            op=OP.subtract)
                    AB.append((A, Bv))
                for t in range(T):
                    for fh in range(nfh):
                        A, Bv = AB[fh]
                        sl = slice(t * P, (t + 1) * P)
                        t1 = wk.tile([P, P], F32, tag="t1", name="t1")
                        nc.vector.tensor_scalar(
                            out=t1[:], in0=v_tiles[fh][:, sl],
                            scalar1=A[:, :1], scalar2=Bv[:, :1],
                            op0=OP.mult, op1=OP.add)
                        t2 = wk.tile([P, P], F32, tag="t2", name="t2")
                        nc.vector.tensor_scalar(
                            out=t2[:], in0=t1[:], scalar1=a_col, scalar2=None,
                            op0=OP.mult)
                        nc.vector.tensor_tensor(
                            out=h_out[fh][:, sl], in0=t1[:], in1=t2[:],
                            op=OP.max)
                return h_out

            h1 = layer([h0], [w1_s], IN, HID, agin1, table1,
                       [gb_s[:, 0:1], gb_s[:, 1:2]],
                       [gb_s[:, 4:5], gb_s[:, 5:6]], a_s[:, 0:1], "hB", False)
            h2 = layer(h1, [w2a_s, w2b_s], HID, OUT, agin2, table2,
                       [gb_s[:, 2:3]], [gb_s[:, 6:7]], a_s[:, 1:2], "hA",
                       False)
            h3 = layer(h2, [wd_s], OUT, IN, agin3, table3,
                       [gb_s[:, 3:4]], [gb_s[:, 7:8]], a_s[:, 2:3], "hB",
                       True)

            rex = h3[0]
            for b in range(T):
                ptb = pstr.tile([P, P], BF16, tag="ptrbf", name="ptrbf")
                nc.tensor.transpose(ptb[:], rex[:, b * P:(b + 1) * P],
                                    idbf_s[:])
                rn = wk.tile([P, P], F32, tag="rn", name="rn")
                nc.vector.tensor_copy(out=rn[:], in_=ptb[:])
                nc.sync.dma_start(rex_dram[b * P:(b + 1) * P, :], rn[:])
            acc = cpool.tile([P, mc], F32, tag="acc", name="acc")
            for m in range(mc):
                pg = gp.tile([P, IN], F32, tag="pg", name="pg")
                nc.gpsimd.indirect_dma_start(
                    out=pg[:], out_offset=None, in_=rex_dram[:],
                    in_offset=IndirectOffsetOnAxis(ap=mrl_s[:, m:m + 1],
                                                   axis=0))
                tg = gp.tile([P, IN], F32, tag="tg", name="tg")
                nc.gpsimd.indirect_dma_start(
                    out=tg[:], out_offset=None, in_=x_full[:],
                    in_offset=IndirectOffsetOnAxis(ap=mrg_s[:, m:m + 1],
                                                   axis=0))
                pp = wk.tile([P, 1], F32, tag="pp", name="pp")
                tt = wk.tile([P, 1], F32, tag="tt", name="tt")
                ptv = wk.tile([P, 1], F32, tag="ptv", name="ptv")
                tmp = wk.tile([P, IN], F32, tag="tmp", name="tmp")
                nc.vector.tensor_tensor(out=tmp[:], in0=pg[:], in1=pg[:],
                                        op=OP.mult)
                nc.vector.tensor_reduce(out=pp[:], in_=tmp[:],
                                        axis=mybir.AxisListType.X, op=OP.add)
                nc.vector.tensor_tensor(out=tmp[:], in0=tg[:], in1=tg[:],
                                        op=OP.mult)
                nc.vector.tensor_reduce(out=tt[:], in_=tmp[:],
                                        axis=mybir.AxisListType.X, op=OP.add)
                nc.vector.tensor_tensor(out=tmp[:], in0=pg[:], in1=tg[:],
                                        op=OP.mult)
                nc.vector.tensor_reduce(out=ptv[:], in_=tmp[:],
                                        axis=mybir.AxisListType.X, op=OP.add)
                q = wk.tile([P, 1], F32, tag="q", name="q")
                nc.vector.tensor_tensor(out=q[:], in0=pp[:], in1=tt[:],
                                        op=OP.mult)
                nc.vector.tensor_scalar(out=q[:], in0=q[:], scalar1=1e-30,
                                        scalar2=None, op0=OP.add)
                rq = wk.tile([P, 1], F32, tag="rq", name="rq")
                nc.scalar.activation(rq[:], q[:],
                                     mybir.ActivationFunctionType.Sqrt)
                nc.vector.reciprocal(rq[:], rq[:])
                nc.vector.tensor_tensor(out=rq[:], in0=ptv[:], in1=rq[:],
                                        op=OP.mult)
                nc.vector.tensor_tensor(out=acc[:, m:m + 1], in0=rq[:],
                                        in1=mv_s[:, m:m + 1], op=OP.mult)
            accr = wk.tile([P, 1], F32, tag="accr", name="accr")
            nc.vector.tensor_reduce(out=accr[:], in_=acc[:],
                                    axis=mybir.AxisListType.X, op=OP.add)
            pl = pstr.tile([1, 1], F32, tag="ptr32", name="ptr32")
            nc.tensor.matmul(pl[:], lhsT=accr[:], rhs=ones_s[:], start=True,
                             stop=True)
            lsb = wk.tile([1, 16], F32, tag="lsb", name="lsb")
            nc.gpsimd.memset(lsb[:], 0.0)
            nc.vector.tensor_copy(out=lsb[:, 0:1], in_=pl[:])
            nc.sync.dma_start(loss_in[:], lsb[:])
            cc3 = nc.gpsimd.collective_compute(
                "AllReduce", OP.add, replica_groups=RG,
                ins=[loss_in[:].opt()], outs=[loss_out[:].opt()])
            lsum = wk.tile([1, 16], F32, tag="lsum", name="lsum")
            ld3 = nc.sync.dma_start(lsum[:], loss_out[:])
            add_dep_helper(ld3.ins, cc3.ins, True, "loss after ar")
            nc.vector.tensor_scalar(out=lsb[:, 0:1], in0=lsum[:, 0:1],
                                    scalar1=-1.0 / NMASK, scalar2=1.0,
                                    op0=OP.mult, op1=OP.add)
            nc.sync.dma_start(out_t[:], lsb[:, 0:1])
    return nc


def prepare(inputs):
    inputs = {k: np.asarray(v) for k, v in inputs.items()}
    edge_index = inputs["edge_index"].astype(np.int64)
    mask_nodes = inputs["mask_nodes"].astype(np.int64)
    x = inputs["x"].astype(np.float32)
    (dinv_pad, srcs, dstl, kc, mlocal, mglob, mvalid, mkeep, mc) = _prep(
        edge_index, mask_nodes)

    nc = build_nc(kc, mc)

    iota = np.broadcast_to(np.arange(P, dtype=np.float32), (P, P)).astype(BF)
    ident_bf = np.eye(P, dtype=np.float32).astype(BF)
    ident_f32 = np.eye(P, dtype=np.float32)
    gb = np.zeros((P, 8), np.float32)
    gb[:, 0] = inputs["g1"][:P]
    gb[:, 1] = inputs["g1"][P:]
    gb[:, 2] = inputs["g2"]
    gb[:, 3] = inputs["gd"]
    gb[:, 4] = inputs["be1"][:P]
    gb[:, 5] = inputs["be1"][P:]
    gb[:, 6] = inputs["be2"]
    gb[:, 7] = inputs["bed"]
    a_rep = np.zeros((P, 3), np.float32)
    a_rep[:, 0] = inputs["a1"][0]
    a_rep[:, 1] = inputs["a2"][0]
    a_rep[:, 2] = inputs["ad"][0]
    w1 = inputs["W1"].astype(BF)
    w2 = inputs["W2"].astype(BF)
    wd = inputs["Wd"].astype(BF)
    token = inputs["mask_token"].astype(np.float32)

    in_maps = []
    for c in range(NC):
        rows = np.arange(c * SHARD, (c + 1) * SHARD)
        xs = np.zeros((SHARD, IN), np.float32)
        v = rows < N
        xs[v] = x[rows[v]]
        mcol = np.ascontiguousarray(
            mkeep[c].reshape(T, P).T)  # [128, T]
        in_maps.append({
            "x_shard": xs,
            "x_full": x,
            "src_rows": srcs[c],
            "dst_local": dstl[c],
            "dinv_rep": np.ascontiguousarray(np.broadcast_to(
                dinv_pad[c * SHARD:(c + 1) * SHARD][None, :], (P, SHARD))),
            "mcol": mcol,
            "token_rep": np.ascontiguousarray(
                np.broadcast_to(token[None, :], (P, IN))),
            "iota_bf": np.ascontiguousarray(iota),
            "ident_bf": ident_bf,
            "ident_f32": ident_f32,
            "ones_col": np.ones((P, 1), np.float32),
            "w1": w1,
            "w2a": np.ascontiguousarray(w2[:P]),
            "w2b": np.ascontiguousarray(w2[P:]),
            "wd": wd,
            "gb": gb,
            "a_rep": a_rep,
            "mrow_l": mlocal[c],
            "mrow_g": mglob[c],
            "mval": mvalid[c],
        })
    return nc, in_maps


def kernel(**inputs):
    import os
    nc, in_maps = prepare(inputs)
    res = run_bass_kernel_spmd(nc, in_maps, core_ids=list(range(NC)),
                               trace=bool(os.environ.get("KTRACE")))
    kernel._last_results = res
    loss = res.results[0]["loss"][0, 0]
    return np.float32(loss).reshape(())



# revision 8
# speedup vs baseline: 3.2067x; 3.2067x over previous
"""GraphMAE-style GNN forward (3x GCNConv + BN + PReLU + SCE loss) on 8 TRN2
NeuronCores via Bass/Tile.

Sharding: nodes partitioned across 8 cores (6272 rows each, node space padded
to 50176). Per layer: sharded dense matmul (feature-major) -> row scale by
dinv -> AllGather of the bf16 node-major table -> per-dst-tile edge
aggregation via indirect-DMA row gathers + one-hot segment-sum matmuls
accumulated in PSUM (self-loops folded in as identity matmuls on the local
shard tile) -> BatchNorm batch stats via free-dim reductions + AllReduce ->
fused BN affine + PReLU (as max(t, a*t), valid for 0<a<=1). Loss: per-core
masked-row cosine-similarity partial sums, AllReduce, 1 - sum/NMASK.

Per-feature conv bias is skipped: training-mode BatchNorm subtracts the batch
mean, so the bias cancels exactly.
"""
import sys

sys.path.insert(0, "/opt/trn_rl_repo")
import numpy as np
import ml_dtypes

import concourse.bass as bass
import concourse.mybir as mybir
import concourse.tile as tile
from bass_rust import SyncInfo
from concourse.bass import IndirectOffsetOnAxis
from concourse.bass_utils import run_bass_kernel_spmd
from concourse.tile_rust import add_dep_helper
from concourse.vector_clock import ScopedClock

BF = ml_dtypes.bfloat16
F32 = mybir.dt.float32
BF16 = mybir.dt.bfloat16
I32 = mybir.dt.int32
OP = mybir.AluOpType

N, E, IN, HID, OUT = 50000, 800000, 128, 256, 128
NC, P = 8, 128
SHARD, T = 6272, 49
NPAD = NC * SHARD
NMASK = 25000
EPS = 1e-5
RG = [list(range(NC))]

# ---------------------------------------------------------------------------
# Walrus on this toolchain rejects >1 semaphore wait per instruction
# ("Too many sync wait commands"). Post-process lowered blocks: hoist excess
# waits onto injected same-engine NoOps (program order => equivalent).
_uid = [0]


def _split_bb_waits(nc):
    for f in nc.m.functions:
        for bb in f.blocks:
            insts = list(bb.instructions)
            out = []
            changed = False
            for inst in insts:
                si = inst.sync_info
                waits = list(si.on_wait) if si is not None and si.on_wait else []
                if len(waits) > 1:
                    changed = True
                    rest = waits[:-1]
                    inst.sync_info.on_wait = waits[-1:]
                    while rest:
                        _uid[0] += 1
                        nop = mybir.InstNoOp(
                            name=f"I-waitsplit-{_uid[0]}", ins=[], outs=[])
                        nop.engine = inst.engine
                        nop.sync_info = SyncInfo(
                            on_wait=rest[:1], on_update=[])
                        rest = rest[1:]
                        out.append(nop)
                out.append(inst)
            if changed:
                bb.instructions = out


class TileContextSplitDrain(tile.TileContext):
    def _drain_and_barrier(self, tick_clock, wait_clock):
        nc = self.nc
        probe = nc.sync.nop(nofuse=True)
        wait_clock.add_sem_waits(
            probe.ins, ScopedClock({None: tick_clock.global_clock}))
        nc.sync.drain()
        nc.all_engine_barrier()
        assert self.sems is not None
        popped = nc._tile_sem_poison_stack.pop()
        assert popped is self._sem_poison
        nc.clear_and_free_semaphores(list(self.sems.allocated().values()))
        nc.all_engine_barrier()
        _split_bb_waits(nc)


# ---------------------------------------------------------------------------
def _prep(edge_index, mask_nodes):
    src = edge_index[0].astype(np.int64)
    dst = edge_index[1].astype(np.int64)
    deg = np.bincount(dst, minlength=N).astype(np.float64) + 1.0
    dinv = (1.0 / np.sqrt(deg)).astype(np.float32)
    dinv_pad = np.zeros(NPAD, np.float32)
    dinv_pad[:N] = dinv

    mask_set = np.zeros(N, bool)
    mask_set[mask_nodes] = True

    pertile = []
    kc = 0
    core_of = dst // SHARD
    for c in range(NC):
        sel = core_of == c
        es, ed = src[sel], dst[sel] - c * SHARD
        tl = ed // P
        order = np.argsort(tl, kind="stable")
        es, ed, tl = es[order], ed[order], tl[order]
        tiles = []
        for t in range(T):
            m = tl == t
            tiles.append((es[m], (ed[m] % P).astype(np.float32)))
            kc = max(kc, (int(m.sum()) + P - 1) // P)
        pertile.append(tiles)

    srcs, dstl = [], []
    for c in range(NC):
        sc = np.zeros((T * kc, P), np.int32)
        dc = np.full((T * kc, P), 255.0, np.float32)
        for t in range(T):
            es, dl = pertile[c][t]
            n = len(es)
            sc[t * kc:(t + 1) * kc].reshape(-1)[:n] = es
            dc[t * kc:(t + 1) * kc].reshape(-1)[:n] = dl
        srcs.append(np.ascontiguousarray(sc.T))
        dstl.append(np.ascontiguousarray(dc.T.astype(BF)))

    mlocal, mglob, mvalid, mkeep = [], [], [], []
    locs = []
    mc = 0
    for c in range(NC):
        rows = np.arange(c * SHARD, (c + 1) * SHARD)
        valid = rows < N
        mm = np.zeros(SHARD, bool)
        mm[:valid.sum()] = mask_set[rows[valid]]
        loc = np.where(mm)[0]
        mc = max(mc, (len(loc) + P - 1) // P)
        locs.append(loc)
        keep = np.ones(SHARD, np.float32)
        keep[mm] = 0.0
        mkeep.append(keep)
    for c in range(NC):
        loc = locs[c]
        n = len(loc)
        lo = np.zeros((mc, P), np.int32)
        gl = np.zeros((mc, P), np.int32)
        va = np.zeros((mc, P), np.float32)
        lo.reshape(-1)[:n] = loc
        gl.reshape(-1)[:n] = loc + c * SHARD
        va.reshape(-1)[:n] = 1.0
        mlocal.append(np.ascontiguousarray(lo.T))
        mglob.append(np.ascontiguousarray(gl.T))
        mvalid.append(np.ascontiguousarray(va.T))
    return dinv_pad, srcs, dstl, kc, mlocal, mglob, mvalid, mkeep, mc


def build_nc(kc, mc):
    nc = bass.Bass(num_devices=NC, target_bir_lowering=False)
    D = {}

    def inp(name, shape, dt):
        D[name] = nc.dram_tensor(name, shape, dt, kind="ExternalInput")
        return D[name]

    x_shard = inp("x_shard", [SHARD, IN], F32)
    inp("src_rows", [P, T * kc], I32)
    inp("dst_local", [P, T * kc], BF16)
    inp("dinv_row", [1, SHARD], F32)
    inp("mcol", [P, T], F32)
    inp("token_rep", [P, IN], F32)
    inp("iota_bf", [P, P], BF16)
    inp("ident_bf", [P, P], BF16)
    inp("ident_f32", [P, P], F32)
    inp("ones_col", [P, 1], F32)
    inp("w1", [IN, HID], BF16)
    inp("w2a", [P, OUT], BF16)
    inp("w2b", [P, OUT], BF16)
    inp("wd", [OUT, IN], BF16)
    inp("gb", [P, 8], F32)
    inp("a_rep", [P, 3], F32)
    inp("mrow_l", [P, mc], I32)
    inp("mval", [P, mc], F32)
    out_t = nc.dram_tensor("loss", [1, 1], F32, kind="ExternalOutput")

    with TileContextSplitDrain(nc) as tc:
        with (
            tc.tile_pool(name="const", bufs=1) as cpool,
            tc.tile_pool(name="hbuf", bufs=1) as hpool,
            tc.tile_pool(name="vbuf", bufs=1) as vpool,
            tc.tile_pool(name="hwn", bufs=1) as hwpool,
            tc.tile_pool(name="work", bufs=2) as wk,
            tc.tile_pool(name="gpool", bufs=4) as gp,
            tc.tile_pool(name="psmm", bufs=2, space="PSUM") as psmm,
            tc.tile_pool(name="pstr", bufs=1, space="PSUM") as pstr,
            tc.tile_pool(name="psagg", bufs=1, space="PSUM") as psagg,
            tc.tile_pool(name="dram", bufs=1, space="DRAM") as dpool,
        ):
            def load(tname):
                h = D[tname]
                t_ = cpool.tile(list(h.shape), h.dtype, tag=tname, name=tname)
                nc.sync.dma_start(t_[:], h[:])
                return t_

            src_s = load("src_rows")
            dstl_s = load("dst_local")
            dinv_s = cpool.tile([P, SHARD], F32, tag="dinv_rep",
                                name="dinv_rep")
            nc.sync.dma_start(dinv_s[:],
                              D["dinv_row"][0:1, :].to_broadcast([P, SHARD]))
            mcol_s = load("mcol")
            tokr_s = load("token_rep")
            iota_s = load("iota_bf")
            idbf_s = load("ident_bf")
            idf_s = load("ident_f32")
            ones_s = load("ones_col")
            w1_s = load("w1")
            w2a_s = load("w2a")
            w2b_s = load("w2b")
            wd_s = load("wd")
            gb_s = load("gb")
            a_s = load("a_rep")
            mrl_s = load("mrow_l")
            mv_s = load("mval")

            # h0: load x shard, mask in node-major, transpose to f-major bf16
            h0 = hpool.tile([P, SHARD], BF16, tag="hA_0", name="hA_0")
            for b in range(T):
                xt = wk.tile([P, IN], F32, tag="xt", name="xt")
                nc.sync.dma_start(xt[:], x_shard[b * P:(b + 1) * P, :])
                nc.vector.tensor_tensor(out=xt[:], in0=xt[:], in1=tokr_s[:],
                                        op=OP.subtract)
                nc.vector.tensor_scalar(out=xt[:], in0=xt[:],
                                        scalar1=mcol_s[:, b:b + 1], scalar2=None,
                                        op0=OP.mult)
                nc.vector.tensor_tensor(out=xt[:], in0=xt[:], in1=tokr_s[:],
                                        op=OP.add)
                pt = pstr.tile([P, P], F32, tag="ptr32", name="ptr32")
                nc.tensor.transpose(pt[:], xt[:], idf_s[:])
                nc.vector.tensor_copy(out=h0[:, b * P:(b + 1) * P], in_=pt[:])

            table1 = dpool.tile([NPAD, HID], BF16, addr_space="Shared", name="table1")
            table2 = dpool.tile([NPAD, OUT], BF16, addr_space="Shared", name="table2")
            table3 = dpool.tile([NPAD, IN], BF16, addr_space="Shared", name="table3")
            agin1 = dpool.tile([SHARD, HID], BF16, name="agin1")
            agin2 = dpool.tile([SHARD, OUT], BF16, name="agin2")
            agin3 = dpool.tile([SHARD, IN], BF16, name="agin3")
            loss_in = dpool.tile([1, 16], F32, name="loss_in")
            loss_out = dpool.tile([1, 16], F32, addr_space="Shared", name="loss_out")
            rex_dram = dpool.tile([SHARD, IN], F32, name="rex_dram")

            def layer(h_tiles, w_tiles, f_in, f_out, agin, table, g_col,
                      be_col, a_col, otag, mask_hwn):
                nfh = f_out // P
                nkt = f_in // P
                hwn = [hwpool.tile([P, f_out], BF16, tag=f"hwn_{b}", name=f"hwn_{b}")
                       for b in range(T)]
                for b in range(T):
                    for fh in range(nfh):
                        pm = psmm.tile([P, P], F32, tag="pm", name="pm")
                        for kt in range(nkt):
                            nc.tensor.matmul(
                                pm[:], lhsT=w_tiles[kt][:, fh * P:(fh + 1) * P],
                                rhs=h_tiles[kt][:, b * P:(b + 1) * P],
                                start=(kt == 0), stop=(kt == nkt - 1))
                        hwT = wk.tile([P, P], BF16, tag="hwT", name="hwT")
                        nc.vector.tensor_tensor(
                            out=hwT[:], in0=pm[:],
                            in1=dinv_s[:, b * P:(b + 1) * P], op=OP.mult)
                        ptb = pstr.tile([P, P], BF16, tag="ptrbf", name="ptrbf")
                        nc.tensor.transpose(ptb[:], hwT[:], idbf_s[:])
                        sl = hwn[b][:, fh * P:(fh + 1) * P]
                        if mask_hwn:
                            nc.vector.tensor_scalar(
                                out=sl, in0=ptb[:], scalar1=mcol_s[:, b:b + 1],
                                scalar2=None, op0=OP.mult)
                        else:
                            nc.vector.tensor_copy(out=sl, in_=ptb[:])
                    nc.sync.dma_start(agin[b * P:(b + 1) * P, :], hwn[b][:])
                cc = nc.gpsimd.collective_compute(
                    "AllGather", OP.bypass, replica_groups=RG,
                    ins=[agin[:].opt()], outs=[table[:].opt()])
                fence = nc.gpsimd.memset(
                    wk.tile([1, 1], F32, tag="fence", name="fence")[:], 0.0)
                add_dep_helper(fence.ins, cc.ins, True, "fence cc")

                v_tiles = [vpool.tile([P, SHARD], BF16, tag=f"v_{fh}", name=f"v_{fh}")
                           for fh in range(nfh)]
                s_cols = [wk.tile([P, T], F32, tag=f"sc_{fh}", name=f"sc_{fh}")
                          for fh in range(nfh)]
                q_cols = [wk.tile([P, T], F32, tag=f"qc_{fh}", name=f"qc_{fh}")
                          for fh in range(nfh)]
                for t in range(T):
                    pa = [psagg.tile([P, P], F32, tag=f"pa{fh}", name=f"pa{fh}")
                          for fh in range(nfh)]
                    for fh in range(nfh):
                        nc.tensor.matmul(
                            pa[fh][:], lhsT=hwn[t][:, fh * P:(fh + 1) * P],
                            rhs=idbf_s[:], start=True, stop=False)
                    for k in range(kc):
                        j = t * kc + k
                        G = gp.tile([P, f_out], BF16, tag="G", name="G")
                        nc.gpsimd.indirect_dma_start(
                            out=G[:], out_offset=None, in_=table[:],
                            in_offset=IndirectOffsetOnAxis(
                                ap=src_s[:, j:j + 1], axis=0))
                        S = gp.tile([P, P], BF16, tag="S", name="S")
                        nc.vector.tensor_tensor(
                            out=S[:],
                            in0=dstl_s[:, j:j + 1].to_broadcast([P, P]),
                            in1=iota_s[:], op=OP.is_equal)
                        for fh in range(nfh):
                            nc.tensor.matmul(
                                pa[fh][:], lhsT=G[:, fh * P:(fh + 1) * P],
                                rhs=S[:], start=False, stop=(k == kc - 1))
                    for fh in range(nfh):
                        vt = v_tiles[fh]
                        sl = slice(t * P, (t + 1) * P)
                        nc.vector.tensor_tensor(
                            out=vt[:, sl], in0=pa[fh][:],
                            in1=dinv_s[:, sl], op=OP.mult)
                        nc.vector.tensor_reduce(
                            out=s_cols[fh][:, t:t + 1], in_=vt[:, sl],
                            axis=mybir.AxisListType.X, op=OP.add)
                        sq = wk.tile([P, P], F32, tag="sq", name="sq")
                        nc.vector.tensor_tensor(
                            out=sq[:], in0=vt[:, sl], in1=vt[:, sl],
                            op=OP.mult)
                        nc.vector.tensor_reduce(
                            out=q_cols[fh][:, t:t + 1], in_=sq[:],
                            axis=mybir.AxisListType.X, op=OP.add)

                stats_in = dpool.tile([P, 4], F32, name=f"stats_in_{otag}")
                stats_out = dpool.tile([P, 4], F32, addr_space="Shared",
                                       name=f"stats_out_{otag}")
                st = wk.tile([P, 4], F32, tag="stats", name="stats")
                nc.gpsimd.memset(st[:], 0.0)
                for fh in range(nfh):
                    nc.vector.tensor_reduce(
                        out=st[:, fh:fh + 1], in_=s_cols[fh][:],
                        axis=mybir.AxisListType.X, op=OP.add)
                    nc.vector.tensor_reduce(
                        out=st[:, 2 + fh:3 + fh], in_=q_cols[fh][:],
                        axis=mybir.AxisListType.X, op=OP.add)
                nc.sync.dma_start(stats_in[:], st[:])
                cc2 = nc.gpsimd.collective_compute(
                    "AllReduce", OP.add, replica_groups=RG,
                    ins=[stats_in[:].opt()], outs=[stats_out[:].opt()])
                st2 = wk.tile([P, 4], F32, tag="stats2", name="stats2")
                ld2 = nc.sync.dma_start(st2[:], stats_out[:])
                add_dep_helper(ld2.ins, cc2.ins, True, "stats after ar")
                h_out = [hpool.tile([P, SHARD], BF16, tag=f"{otag}_{fh}", name=f"{otag}_{fh}")
                         for fh in range(nfh)]
                AB = []
                for fh in range(nfh):
                    mu = wk.tile([P, 1], F32, tag=f"mu{fh}", name=f"mu{fh}")
                    nc.vector.tensor_scalar(out=mu[:], in0=st2[:, fh:fh + 1],
                                            scalar1=1.0 / N, scalar2=None,
                                            op0=OP.mult)
                    var = wk.tile([P, 1], F32, tag=f"var{fh}", name=f"var{fh}")
                    nc.vector.tensor_tensor(out=var[:], in0=mu[:], in1=mu[:],
                                            op=OP.mult)
                    ms = wk.tile([P, 1], F32, tag=f"ms{fh}", name=f"ms{fh}")
                    nc.vector.tensor_scalar(out=ms[:], in0=st2[:, 2 + fh:3 + fh],
                                            scalar1=1.0 / N, scalar2=None,
                                            op0=OP.mult)
                    nc.vector.tensor_tensor(out=var[:], in0=ms[:], in1=var[:],
                                            op=OP.subtract)
                    nc.vector.tensor_scalar(out=var[:], in0=var[:], scalar1=EPS,
                                            scalar2=None, op0=OP.add)
                    rs = wk.tile([P, 1], F32, tag=f"rs{fh}", name=f"rs{fh}")
                    nc.scalar.activation(rs[:], var[:],
                                         mybir.ActivationFunctionType.Sqrt)
                    nc.vector.reciprocal(rs[:], rs[:])
                    A = wk.tile([P, 1], F32, tag=f"A{fh}", name=f"A{fh}")
                    nc.vector.tensor_tensor(out=A[:], in0=g_col[fh], in1=rs[:],
                                            op=OP.mult)
                    Bv = wk.tile([P, 1], F32, tag=f"B{fh}", name=f"B{fh}")
                    nc.vector.tensor_tensor(out=Bv[:], in0=mu[:], in1=A[:],
                                            op=OP.mult)
                    nc.vector.tensor_tensor(out=Bv[:], in0=be_col[fh], in1=Bv[:],
                                            op=OP.subtract)
                    AB.append((A, Bv))
                for t in range(T):
                    for fh in range(nfh):
                        A, Bv = AB[fh]
                        sl = slice(t * P, (t + 1) * P)
                        t1 = wk.tile([P, P], F32, tag="t1", name="t1")
                        nc.vector.tensor_scalar(
                            out=t1[:], in0=v_tiles[fh][:, sl],
                            scalar1=A[:, :1], scalar2=Bv[:, :1],
                            op0=OP.mult, op1=OP.add)
                        t2 = wk.tile([P, P], F32, tag="t2", name="t2")
                        nc.vector.tensor_scalar(
                            out=t2[:], in0=t1[:], scalar1=a_col, scalar2=None,
                            op0=OP.mult)
                        nc.vector.tensor_tensor(
                            out=h_out[fh][:, sl], in0=t1[:], in1=t2[:],
                            op=OP.max)
                return h_out

            h1 = layer([h0], [w1_s], IN, HID, agin1, table1,
                       [gb_s[:, 0:1], gb_s[:, 1:2]],
                       [gb_s[:, 4:5], gb_s[:, 5:6]], a_s[:, 0:1], "hB", False)
            h2 = layer(h1, [w2a_s, w2b_s], HID, OUT, agin2, table2,
                       [gb_s[:, 2:3]], [gb_s[:, 6:7]], a_s[:, 1:2], "hA",
                       False)
            h3 = layer(h2, [wd_s], OUT, IN, agin3, table3,
                       [gb_s[:, 3:4]], [gb_s[:, 7:8]], a_s[:, 2:3], "hB",
                       True)

            rex = h3[0]
            for b in range(T):
                ptb = pstr.tile([P, P], BF16, tag="ptrbf", name="ptrbf")
                nc.tensor.transpose(ptb[:], rex[:, b * P:(b + 1) * P],
                                    idbf_s[:])
                rn = wk.tile([P, P], F32, tag="rn", name="rn")
                nc.vector.tensor_copy(out=rn[:], in_=ptb[:])
                nc.sync.dma_start(rex_dram[b * P:(b + 1) * P, :], rn[:])
            acc = cpool.tile([P, mc], F32, tag="acc", name="acc")
            for m in range(mc):
                pg = gp.tile([P, IN], F32, tag="pg", name="pg")
                nc.gpsimd.indirect_dma_start(
                    out=pg[:], out_offset=None, in_=rex_dram[:],
                    in_offset=IndirectOffsetOnAxis(ap=mrl_s[:, m:m + 1],
                                                   axis=0))
                tg = gp.tile([P, IN], F32, tag="tg", name="tg")
                nc.gpsimd.indirect_dma_start(
                    out=tg[:], out_offset=None, in_=x_shard[:],
                    in_offset=IndirectOffsetOnAxis(ap=mrl_s[:, m:m + 1],
                                                   axis=0))
                pp = wk.tile([P, 1], F32, tag="pp", name="pp")
                tt = wk.tile([P, 1], F32, tag="tt", name="tt")
                ptv = wk.tile([P, 1], F32, tag="ptv", name="ptv")
                tmp = wk.tile([P, IN], F32, tag="tmp", name="tmp")
                nc.vector.tensor_tensor(out=tmp[:], in0=pg[:], in1=pg[:],
                                        op=OP.mult)
                nc.vector.tensor_reduce(out=pp[:], in_=tmp[:],
                                        axis=mybir.AxisListType.X, op=OP.add)
                nc.vector.tensor_tensor(out=tmp[:], in0=tg[:], in1=tg[:],
                                        op=OP.mult)
                nc.vector.tensor_reduce(out=tt[:], in_=tmp[:],
                                        axis=mybir.AxisListType.X, op=OP.add)
                nc.vector.tensor_tensor(out=tmp[:], in0=pg[:], in1=tg[:],
                                        op=OP.mult)
                nc.vector.tensor_reduce(out=ptv[:], in_=tmp[:],
                                        axis=mybir.AxisListType.X, op=OP.add)
                q = wk.tile([P, 1], F32, tag="q", name="q")
                nc.vector.tensor_tensor(out=q[:], in0=pp[:], in1=tt[:],
                                        op=OP.mult)
                nc.vector.tensor_scalar(out=q[:], in0=q[:], scalar1=1e-30,
                                        scalar2=None, op0=OP.add)
                rq = wk.tile([P, 1], F32, tag="rq", name="rq")
                nc.scalar.activation(rq[:], q[:],
                                     mybir.ActivationFunctionType.Sqrt)
                nc.vector.reciprocal(rq[:], rq[:])
                nc.vector.tensor_tensor(out=rq[:], in0=ptv[:], in1=rq[:],
                                        op=OP.mult)
                nc.vector.tensor_tensor(out=acc[:, m:m + 1], in0=rq[:],
                                        in1=mv_s[:, m:m + 1], op=OP.mult)
            accr = wk.tile([P, 1], F32, tag="accr", name="accr")
            nc.vector.tensor_reduce(out=accr[:], in_=acc[:],
                                    axis=mybir.AxisListType.X, op=OP.add)
            pl = pstr.tile([1, 1], F32, tag="ptr32", name="ptr32")
            nc.tensor.matmul(pl[:], lhsT=accr[:], rhs=ones_s[:], start=True,
                             stop=True)
            lsb = wk.tile([1, 16], F32, tag="lsb", name="lsb")
            nc.gpsimd.memset(lsb[:], 0.0)
            nc.vector.tensor_copy(out=lsb[:, 0:1], in_=pl[:])
            nc.sync.dma_start(loss_in[:], lsb[:])
            cc3 = nc.gpsimd.collective_compute(
                "AllReduce", OP.add, replica_groups=RG,
                ins=[loss_in[:].opt()], outs=[loss_out[:].opt()])
            lsum = wk.tile([1, 16], F32, tag="lsum", name="lsum")
            ld3 = nc.sync.dma_start(lsum[:], loss_out[:])
            add_dep_helper(ld3.ins, cc3.ins, True, "loss after ar")
            nc.vector.tensor_scalar(out=lsb[:, 0:1], in0=lsum[:, 0:1],
                                    scalar1=-1.0 / NMASK, scalar2=1.0,
                                    op0=OP.mult, op1=OP.add)
            nc.sync.dma_start(out_t[:], lsb[:, 0:1])
    return nc


def prepare(inputs):
    inputs = {k: np.asarray(v) for k, v in inputs.items()}
    edge_index = inputs["edge_index"].astype(np.int64)
    mask_nodes = inputs["mask_nodes"].astype(np.int64)
    x = inputs["x"].astype(np.float32)
    (dinv_pad, srcs, dstl, kc, mlocal, mglob, mvalid, mkeep, mc) = _prep(
        edge_index, mask_nodes)

    nc = build_nc(kc, mc)

    iota = np.broadcast_to(np.arange(P, dtype=np.float32), (P, P)).astype(BF)
    ident_bf = np.eye(P, dtype=np.float32).astype(BF)
    ident_f32 = np.eye(P, dtype=np.float32)
    gb = np.zeros((P, 8), np.float32)
    gb[:, 0] = inputs["g1"][:P]
    gb[:, 1] = inputs["g1"][P:]
    gb[:, 2] = inputs["g2"]
    gb[:, 3] = inputs["gd"]
    gb[:, 4] = inputs["be1"][:P]
    gb[:, 5] = inputs["be1"][P:]
    gb[:, 6] = inputs["be2"]
    gb[:, 7] = inputs["bed"]
    a_rep = np.zeros((P, 3), np.float32)
    a_rep[:, 0] = inputs["a1"][0]
    a_rep[:, 1] = inputs["a2"][0]
    a_rep[:, 2] = inputs["ad"][0]
    w1 = inputs["W1"].astype(BF)
    w2 = inputs["W2"].astype(BF)
    wd = inputs["Wd"].astype(BF)
    token = inputs["mask_token"].astype(np.float32)

    in_maps = []
    for c in range(NC):
        rows = np.arange(c * SHARD, (c + 1) * SHARD)
        xs = np.zeros((SHARD, IN), np.float32)
        v = rows < N
        xs[v] = x[rows[v]]
        mcol = np.ascontiguousarray(
            mkeep[c].reshape(T, P).T)  # [128, T]
        in_maps.append({
            "x_shard": xs,
            "src_rows": srcs[c],
            "dst_local": dstl[c],
            "dinv_row": np.ascontiguousarray(
                dinv_pad[c * SHARD:(c + 1) * SHARD][None, :]),
            "mcol": mcol,
            "token_rep": np.ascontiguousarray(
                np.broadcast_to(token[None, :], (P, IN))),
            "iota_bf": np.ascontiguousarray(iota),
            "ident_bf": ident_bf,
            "ident_f32": ident_f32,
            "ones_col": np.ones((P, 1), np.float32),
            "w1": w1,
            "w2a": np.ascontiguousarray(w2[:P]),
            "w2b": np.ascontiguousarray(w2[P:]),
            "wd": wd,
            "gb": gb,
            "a_rep": a_rep,
            "mrow_l": mlocal[c],
            "mval": mvalid[c],
        })
    return nc, in_maps


def kernel(**inputs):
    import os
    nc, in_maps = prepare(inputs)
    res = run_bass_kernel_spmd(nc, in_maps, core_ids=list(range(NC)),
                               trace=bool(os.environ.get("KTRACE")))
    kernel._last_results = res
    loss = res.results[0]["loss"][0, 0]
    return np.float32(loss).reshape(())

